# revision 31
# baseline (speedup 1.0000x reference)
"""Type-2 NUFFT (image -> non-uniform k-space) on 8 Trainium2 NeuronCores.

kspace[b,m] = sum_{x,y} image[b,x,y] * exp(-i*(kx_m*(x-128) + ky_m*(y-128)))

Quarter-fold decomposition with half-integer centering: write
x-128 = v - 1/2 with v = x - 127.5 in +-{0.5, ..., 127.5}, and likewise
y-128 = u - 1/2.  Then

  kspace[b,m] = e^{i(kx+ky)/2} * sum_{v,u} img * e^{-i(kx v + ky u)}

and the inner sum folds EXACTLY into 128x128 quadrant images (cos is even,
sin is odd in both v and u):

  inner = R - i*N
  R[m] = sum_w C~E[m,w]*cosY[m,w] - S~O[m,w]*sinY[m,w]
  N[m] = sum_w C~O[m,w]*sinY[m,w] + S~E[m,w]*cosY[m,w]
  C~E = cosX^T @ imgEE   C~O = cosX^T @ imgEO      (per batch)
  S~E = sinX^T @ imgOE   S~O = sinX^T @ imgOO

Work split: the host (numpy) computes the trig tables (cosX/sinX per m-tile,
cosY/sinY per m-tile) and the folded quadrant images in bf16, packs them
into one consumption-ordered blob, and applies the final e^{i(kx+ky)/2}
rotation.  The device does the O(M*N^2) work only: per (batch, m-tile) two
bf16 matmuls into PSUM and two fused DVE multiply+row-reduce ops that
accumulate straight into the output column.
"""

import sys

if '/opt/trn_rl_repo' not in sys.path:
    sys.path.insert(0, '/opt/trn_rl_repo')

import numpy as np
import ml_dtypes

B, NX, NY, M, NCORES = 2, 256, 256, 16384, 8
ML = M // NCORES            # 2048 m-points per core
NT = ML // 128              # 16 m-tiles per core

# blob layout (bf16, per partition-col), ordered by first consumption:
#   [img b0 (512) | t0 tables (512) | img b1 (512) | t1..t15 tables]
# where a t-table block is [cx(128) | sx(128) | w=cosY|sinY (256)].
TSTRIDE = 512
BLOB_COLS = 1536 + (NT - 1) * 512


def _tcol(t):
    """blob column where tile t's table block starts"""
    return 512 if t == 0 else 1536 + (t - 1) * TSTRIDE


def _imgcol(b):
    return 0 if b == 0 else 1024

_CACHE = {}


_C = {1, 7, 13, 18, 23, 27, 29}
_B = {3, 9, 15, 21, 26}
DEFAULT_PATTERN = ''.join('C' if i in _C else 'B' if i in _B else 'A'
                          for i in range(32))


def _build(pattern=DEFAULT_PATTERN, psum_bufs=6, work_bufs=10, nchunks=None,
           out_every=8, out_marks=None):
    import concourse.bacc as bacc
    import concourse.bass as bass
    import concourse.mybir as mybir
    from concourse.tile import TileContext

    A = mybir.AluOpType
    f32 = mybir.dt.float32
    bf16 = mybir.dt.bfloat16

    nc = bacc.Bacc("TRN2", target_bir_lowering=False, debug=False)

    blob = nc.dram_tensor("blob", [128, BLOB_COLS], bf16, kind="ExternalInput")
    out = nc.dram_tensor("out", [128, 4 * NT], f32, kind="ExternalOutput")

    def seg2(tile_ap, start, seg_stride):
        """[128, 2, 128] view: two 128-wide segments at start, start+stride."""
        t_ = tile_ap.tensor
        row = tile_ap.ap[0][0]
        return bass.AP(t_, tile_ap.offset + start,
                       [[row, 128], [seg_stride, 2], [1, 128]])

    # DMA chunks over the blob, in consumption order (first small, for a
    # fast pipeline start)
    if nchunks is None:
        bounds = [0, 1024, 1536, _tcol(2), _tcol(4), _tcol(7), _tcol(10),
                  _tcol(13), BLOB_COLS]
    else:
        bounds = nchunks

    with TileContext(nc) as tc:
        with tc.tile_pool(name="const", bufs=1) as cpool, \
             tc.tile_pool(name="work", bufs=work_bufs) as wpool, \
             tc.tile_pool(name="ps", bufs=psum_bufs, space="PSUM") as ps:

            bsb = cpool.tile([128, BLOB_COLS], bf16, name="blob")
            out_sb = cpool.tile([128, 4 * NT], f32)

            for i in range(len(bounds) - 1):
                cs = slice(bounds[i], bounds[i + 1])
                nc.sync.dma_start(bsb[:, cs], blob[:, cs])

            # per-(t,b) stage-2 path: A = DVE fused multiply+reduce from PSUM;
            # B = Act evicts PSUM->SBUF, Pool multiplies, DVE reduces (2x);
            # C = like B but Act reduces.  Balances DVE/Act/Pool busy time.
            PATTERN = pattern
            F = mybir.ActivationFunctionType

            for t in range(NT):
                c0 = _tcol(t)
                for b in range(B):
                    i0 = _imgcol(b)
                    ab = ps.tile([128, 512], f32, tag="ab")
                    # ab = [C~E | C~O | S~E | -S~O]
                    nc.tensor.matmul(ab[:, 0:256],
                                     bsb[:, c0:c0 + 128],
                                     bsb[:, i0:i0 + 256],
                                     start=True, stop=True)
                    nc.tensor.matmul(ab[:, 256:512],
                                     bsb[:, c0 + 128:c0 + 256],
                                     bsb[:, i0 + 256:i0 + 512],
                                     start=True, stop=True)
                    col = t * 4 + b * 2
                    path = PATTERN[t * 2 + b]
                    # R = sum(C~E*cosY) + sum(-S~O*sinY)   -> col
                    # N = sum(C~O*sinY) + sum(S~E*cosY)    -> col+1
                    if path == 'A':
                        scr = wpool.tile([128, 256], f32, tag="scr")
                        scr2 = wpool.tile([128, 256], f32, tag="scr2")
                        nc.vector.scalar_tensor_tensor(
                            seg2(scr[:, :], 0, 128),
                            seg2(ab[:, :], 0, 384), 1.0,
                            seg2(bsb[:, :], c0 + 256, 128),
                            op0=A.mult, op1=A.mult,
                            accum_out=out_sb[:, col:col + 1])
                        nc.vector.scalar_tensor_tensor(
                            seg2(scr2[:, :], 0, 128),
                            seg2(ab[:, :], 128, 128), 1.0,
                            seg2(bsb[:, :], c0 + 384, -128),
                            op0=A.mult, op1=A.mult,
                            accum_out=out_sb[:, col + 1:col + 2])
                    else:
                        cp = wpool.tile([128, 512], f32, tag="cp")
                        if path in 'DE':
                            # DMA engines evict PSUM->SBUF (no engine time)
                            nc.sync.dma_start(cp[:, :], ab[:, :])
                        else:
                            nc.scalar.copy(cp[:, :], ab[:, :])
                        p1 = wpool.tile([128, 256], f32, tag="p1")
                        nc.gpsimd.tensor_tensor(
                            seg2(p1[:, :], 0, 128),
                            seg2(cp[:, :], 0, 384),
                            seg2(bsb[:, :], c0 + 256, 128), op=A.mult)
                        if path in 'FG':
                            # R: reduce Pool's product (DVE 2x ts or Act);
                            # N: DVE fused stt on the SBUF copy
                            d1 = wpool.tile([128, 256], f32, tag="d1")
                            if path == 'F':
                                nc.vector.tensor_scalar(
                                    d1[:, :], p1[:, :], scalar1=1.0,
                                    scalar2=0.0, op0=A.mult, op1=A.add,
                                    accum_out=out_sb[:, col:col + 1])
                            else:
                                nc.scalar.activation(
                                    d1[:, :], p1[:, :], F.Copy,
                                    accum_out=out_sb[:, col:col + 1])
                            scr2 = wpool.tile([128, 256], f32, tag="scr2")
                            nc.vector.scalar_tensor_tensor(
                                seg2(scr2[:, :], 0, 128),
                                seg2(cp[:, :], 128, 128), 1.0,
                                seg2(bsb[:, :], c0 + 384, -128),
                                op0=A.mult, op1=A.mult,
                                accum_out=out_sb[:, col + 1:col + 2])
                            continue
                        p2 = wpool.tile([128, 256], f32, tag="p2")
                        nc.gpsimd.tensor_tensor(
                            seg2(p2[:, :], 0, 128),
                            seg2(cp[:, :], 128, 128),
                            seg2(bsb[:, :], c0 + 384, -128), op=A.mult)
                        if path in 'BEP':
                            eng2 = nc.gpsimd if path == 'P' else nc.vector
                            d1 = wpool.tile([128, 256], f32, tag="d1")
                            d2 = wpool.tile([128, 256], f32, tag="d2")
                            eng2.tensor_scalar(
                                d1[:, :], p1[:, :], scalar1=1.0, scalar2=0.0,
                                op0=A.mult, op1=A.add,
                                accum_out=out_sb[:, col:col + 1])
                            eng2.tensor_scalar(
                                d2[:, :], p2[:, :], scalar1=1.0, scalar2=0.0,
                                op0=A.mult, op1=A.add,
                                accum_out=out_sb[:, col + 1:col + 2])
                        else:
                            d1 = wpool.tile([128, 256], f32, tag="d1")
                            d2 = wpool.tile([128, 256], f32, tag="d2")
                            nc.scalar.activation(
                                d1[:, :], p1[:, :], F.Copy,
                                accum_out=out_sb[:, col:col + 1])
                            nc.scalar.activation(
                                d2[:, :], p2[:, :], F.Copy,
                                accum_out=out_sb[:, col + 1:col + 2])
                marks = (out_marks if out_marks is not None
                         else list(range(out_every - 1, NT, out_every)))
                if t in marks:
                    prev = max([m for m in marks if m < t], default=-1)
                    qs = slice((prev + 1) * 4, (t + 1) * 4)
                    nc.sync.dma_start(out[:, qs], out_sb[:, qs])

    nc.compile()
    return nc


def _host_prep(image, trajectory):
    """Folded quadrant images + trig tables (bf16) packed per-core blobs."""
    bf = ml_dtypes.bfloat16
    kx = trajectory[0].astype(np.float32)            # [M]
    ky = trajectory[1].astype(np.float32)
    v = (np.arange(128, dtype=np.float32) + 0.5)

    cosX = np.cos(kx[None, :] * v[:, None])          # [128, M]
    sinX = np.sin(kx[None, :] * v[:, None])
    argY = ky[:, None] * v[None, :]                  # [M, 128]
    cosY = np.cos(argY)
    sinY = np.sin(argY)

    # quadrant folds (x: rows about 127.5; y: cols about 127.5)
    top = image[:, 128:256, :]
    bot = image[:, 127::-1, :]
    sumx = top + bot
    difx = top - bot
    imgEE = sumx[:, :, 128:256] + sumx[:, :, 127::-1]
    imgEO = sumx[:, :, 128:256] - sumx[:, :, 127::-1]
    imgOE = difx[:, :, 128:256] + difx[:, :, 127::-1]
    imgOOn = difx[:, :, 127::-1] - difx[:, :, 128:256]   # = -imgOO
    imgq = np.concatenate([imgEE, imgEO, imgOE, imgOOn], axis=2)  # [B,128,512]

    # per-core blob [128, BLOB_COLS]
    cx = cosX.reshape(128, NCORES, NT, 128)          # [j, c, t, p]
    sx = sinX.reshape(128, NCORES, NT, 128)
    cy = cosY.reshape(NCORES, NT, 128, 128)          # [c, t, p, w]
    sy = sinY.reshape(NCORES, NT, 128, 128)

    blobs = np.empty((NCORES, 128, BLOB_COLS), dtype=bf)
    blobs[:, :, 0:512] = imgq[0].astype(bf)[None]
    blobs[:, :, 1024:1536] = imgq[1].astype(bf)[None]
    # tables: for core c, tile t: cols [cx_t | sx_t | cy_t | sy_t]
    tbl = np.concatenate([
        cx.transpose(1, 2, 0, 3),                    # [c, t, j, p] -> cx block
        sx.transpose(1, 2, 0, 3),
        cy.transpose(0, 1, 2, 3),                    # [c, t, p, w]
        sy.transpose(0, 1, 2, 3),
    ], axis=-1).astype(bf)                            # [c, t, 128, 512]
    tblp = tbl.transpose(0, 2, 1, 3)                  # [c, 128, t, 512]
    blobs[:, :, 512:1024] = tblp[:, :, 0, :]
    blobs[:, :, 1536:] = tblp[:, :, 1:, :].reshape(NCORES, 128,
                                                   (NT - 1) * TSTRIDE)

    phase = np.exp(1j * (kx + ky) / 2.0).astype(np.complex64)
    return blobs, phase


def kernel(image, trajectory):
    from concourse.bass_utils import run_bass_kernel_spmd

    if 'nc' not in _CACHE:
        _CACHE['nc'] = _build()
    nc = _CACHE['nc']

    image = np.ascontiguousarray(np.asarray(image, dtype=np.float32))
    trajectory = np.ascontiguousarray(np.asarray(trajectory, dtype=np.float32))
    blobs, phase = _host_prep(image, trajectory)

    in_maps = [{"blob": np.ascontiguousarray(blobs[c])} for c in range(NCORES)]

    res = run_bass_kernel_spmd(nc, in_maps, core_ids=list(range(NCORES)))

    kspace = np.empty((B, M), dtype=np.complex64)
    for c in range(NCORES):
        o = res.results[c]["out"]          # [128, 4*NT]
        o = o.reshape(128, NT, 2, 2)       # [p, t, b, (R, N)]
        for b in range(B):
            R = o[:, :, b, 0].T.reshape(ML)    # m = t*128 + p
            N = o[:, :, b, 1].T.reshape(ML)
            kspace[b, c * ML:(c + 1) * ML] = R - 1j * N
    kspace *= phase[None, :]
    return kspace
